# revision 1
# baseline (speedup 1.0000x reference)
"""Trainium2 Bass kernel for nn_MultiHeadAttention_2963527434617.

Math (per token, feature dim D=1024, 16 head-groups of 64 channels):
    Q = elu(q @ wq + bq) + 1
    K = elu(k @ wk + bk) + 1
    V = v @ wv + bv
    Kc = blockwise cumsum of K over the 16 head groups (axis=heads)
    A = Q * (K / Kc) * V            (purely elementwise)
    out = A @ wc + bc

Everything is per-token, so we shard the 32768 tokens across the 8 cores
(4096 tokens each) with zero communication.  The host feeds each core its
token slice pre-transposed to feature-major [D, tok] (bf16) so the device
never has to transpose activations for the matmuls; intermediates are kept
token-major in SBUF so the head cumsum is a cheap free-axis running sum.
A single PE transpose of A precedes the output projection.  Matmul operands
are bf16 with fp32 PSUM accumulation; elementwise math is fp32.
"""

import sys

sys.path.insert(0, "/opt/trn_rl_repo")

import numpy as np
import ml_dtypes

B, L, D_MODEL, N_HEADS = 4, 8192, 1024, 16
D_HEAD = D_MODEL // N_HEADS  # 64
N_CORES = 8
TOKENS = B * L  # 32768
TOK_PER_CORE = TOKENS // N_CORES  # 4096
P = 128
KO = D_MODEL // P  # 8 k-blocks
T_CHUNK = 256  # tokens per pipeline chunk
N_CHUNKS = TOK_PER_CORE // T_CHUNK  # 16
MB = T_CHUNK // P  # 2 token-blocks per chunk
N_HALF = 512  # mm1 moving width (one fp32 PSUM bank)

_BF16 = ml_dtypes.bfloat16

_module_cache = {}


def _build_module(with_bias: bool, repeat: int = 1):
    import contextlib

    import concourse.mybir as mybir
    import concourse.tile as tile
    from concourse import bacc
    from concourse.masks import make_identity

    f32 = mybir.dt.float32
    bf16 = mybir.dt.bfloat16
    AF = mybir.ActivationFunctionType
    OP = mybir.AluOpType

    nc = bacc.Bacc("TRN2", target_bir_lowering=False, debug=False)

    xq_d = nc.dram_tensor("xq_t", (D_MODEL, TOK_PER_CORE), bf16, kind="ExternalInput")
    xk_d = nc.dram_tensor("xk_t", (D_MODEL, TOK_PER_CORE), bf16, kind="ExternalInput")
    xv_d = nc.dram_tensor("xv_t", (D_MODEL, TOK_PER_CORE), bf16, kind="ExternalInput")
    w_d = {
        name: nc.dram_tensor(name, (D_MODEL, D_MODEL), bf16, kind="ExternalInput")
        for name in ("wq", "wk", "wv", "wc")
    }
    out_d = nc.dram_tensor("out_t", (TOK_PER_CORE, D_MODEL), f32, kind="ExternalOutput")

    b_d = {}
    if with_bias:
        # replicated across partitions for token-major free-axis adds
        for name in ("bq", "bk", "bv", "bc"):
            b_d[name] = nc.dram_tensor(
                f"{name}_rep", (P, D_MODEL), f32, kind="ExternalInput"
            )

    xq_r = xq_d.rearrange("(ko p) t -> p ko t", p=P)
    xk_r = xk_d.rearrange("(ko p) t -> p ko t", p=P)
    xv_r = xv_d.rearrange("(ko p) t -> p ko t", p=P)

    with tile.TileContext(nc) as tc:
        with (
            tc.tile_pool(name="const", bufs=1) as constp,
            tc.tile_pool(name="xin", bufs=2) as xinp,
            tc.tile_pool(name="work", bufs=2) as workp,
            tc.tile_pool(name="small", bufs=4) as smallp,
            tc.tile_pool(name="psum1", bufs=4, space="PSUM") as pp1,
            tc.tile_pool(name="psumT", bufs=2, space="PSUM") as ppT,
            tc.tile_pool(name="psum2", bufs=2, space="PSUM") as pp2,
        ):
            # Weights resident in SBUF; loaded on the gpsimd DMA queue so the
            # first chunk's activation loads (sync queue) are not stuck
            # behind 8 MB of weights in the same FIFO.
            w_sb = {}
            for name in ("wq", "wk", "wv", "wc"):
                t = constp.tile([P, KO, D_MODEL], bf16, tag=f"{name}_sb")
                w_r = w_d[name].rearrange("(ko p) n -> p ko n", p=P)
                if name == "wq":
                    # first weight used: split across two SW-DGE queues so
                    # the first matmuls aren't stuck behind one serial 2 MB
                    # DMA (scalar + gpsimd queues run in parallel)
                    nc.scalar.dma_start(t[:, : KO // 2], w_r[:, : KO // 2])
                    nc.gpsimd.dma_start(t[:, KO // 2 :], w_r[:, KO // 2 :])
                else:
                    nc.gpsimd.dma_start(t[:], w_r)
                w_sb[name] = t
            ident = constp.tile([P, P], bf16, tag="ident")
            make_identity(nc, ident)

            b_sb = {}
            if with_bias:
                for name in ("bq", "bk", "bv", "bc"):
                    t = constp.tile([P, D_MODEL], f32, tag=f"{name}_sb")
                    nc.gpsimd.dma_start(t[:], b_d[name][:])
                    b_sb[name] = t

            def proj(x_t, w, elu, dst, bias):
                """dst[:, mb, :] (token-major [P, MB, D]) = act(x @ W + b).

                Loop order mb -> ko -> nh: the stationary operand (a 128x128
                token block of x^T) is reused for both n-halves, halving
                weight loads; the two accumulating PSUM tiles live across the
                ko loop.
                """
                NH = D_MODEL // N_HALF
                for mb in range(MB):
                    pss = [pp1.tile([P, N_HALF], f32, tag="ps1", name=f"ps1_{mb}_{i}") for i in range(NH)]
                    for ko in range(KO):
                        for nh in range(NH):
                            nc.tensor.matmul(
                                pss[nh][:],
                                lhsT=x_t[:, ko, mb * P : (mb + 1) * P],
                                rhs=w[:, ko, nh * N_HALF : (nh + 1) * N_HALF],
                                start=(ko == 0),
                                stop=(ko == KO - 1),
                            )
                    for nh in range(NH):
                        ps = pss[nh]
                        dslice = dst[:, mb, nh * N_HALF : (nh + 1) * N_HALF]
                        src = ps[:]
                        if bias is not None:
                            tmp = smallp.tile([P, N_HALF], f32, tag="btmp")
                            nc.vector.tensor_tensor(
                                tmp[:],
                                ps[:],
                                bias[:, nh * N_HALF : (nh + 1) * N_HALF],
                                OP.add,
                            )
                            src = tmp[:]
                        if elu:
                            # elu(x)+1 == relu(x) + min(exp(x), 1)  (exact)
                            e = smallp.tile([P, N_HALF], f32, tag="e")
                            nc.scalar.activation(dslice, src, AF.Relu)
                            nc.scalar.activation(e[:], src, AF.Exp)
                            nc.vector.tensor_scalar_min(e[:], e[:], 1.0)
                            nc.vector.tensor_tensor(dslice, dslice, e[:], OP.add)
                        else:
                            nc.scalar.activation(dslice, src, AF.Copy)

            def post_block(t0, Q, K, V, mb0, mbn, suffix):
                """Elementwise + transpose + output projection for token
                blocks [mb0, mb0+mbn) of a chunk (token-major tiles)."""
                ms = slice(mb0, mb0 + mbn)
                # head-group cumsum along the free (feature) axis.
                # Kc holds heads 1..15 only (head 0: K/Kc == 1).
                Kc = workp.tile(
                    [P, mbn, D_MODEL - D_HEAD], f32, tag="Kc",
                    name=f"Kc{suffix}",
                )
                nc.vector.tensor_tensor(
                    Kc[:, :, 0:D_HEAD],
                    K[:, ms, 0:D_HEAD],
                    K[:, ms, D_HEAD : 2 * D_HEAD],
                    OP.add,
                )
                for h in range(2, N_HEADS):
                    nc.vector.tensor_tensor(
                        Kc[:, :, (h - 1) * D_HEAD : h * D_HEAD],
                        Kc[:, :, (h - 2) * D_HEAD : (h - 1) * D_HEAD],
                        K[:, ms, h * D_HEAD : (h + 1) * D_HEAD],
                        OP.add,
                    )
                # Kc := 1/Kc  (Kc strictly positive: sums of elu(x)+1 > 0)
                nc.vector.reciprocal_approx_fast(Kc[:], Kc[:])
                # Q := Q * V
                nc.vector.tensor_tensor(Q[:, ms], Q[:, ms], V[:, ms], OP.mult)
                # K := K * (1/Kc) for heads 1..15
                nc.vector.tensor_tensor(
                    K[:, ms, D_HEAD:], K[:, ms, D_HEAD:], Kc[:], OP.mult
                )
                # A = Q*V * (K/Kc), bf16 for the output projection
                A = workp.tile(
                    [P, mbn, D_MODEL], bf16, tag="A", name=f"A{suffix}"
                )
                nc.vector.tensor_copy(A[:, :, 0:D_HEAD], Q[:, ms, 0:D_HEAD])
                nc.vector.tensor_tensor(
                    A[:, :, D_HEAD:], Q[:, ms, D_HEAD:], K[:, ms, D_HEAD:], OP.mult
                )

                # transpose A to feature-major for the output projection
                AT = workp.tile(
                    [P, KO, mbn * P], bf16, tag="AT", name=f"AT{suffix}"
                )
                for fb in range(KO):
                    for mb in range(mbn):
                        pt = ppT.tile([P, P], bf16, tag="pT")
                        nc.tensor.transpose(
                            pt[:], A[:, mb, fb * P : (fb + 1) * P], ident[:]
                        )
                        nc.any.tensor_copy(AT[:, fb, mb * P : (mb + 1) * P], pt[:])

                # out = A @ wc, token-major: lhsT = feature-major A^T block
                # (stationary, reused for both n-halves), rhs = wc rows.
                NH = D_MODEL // N_HALF
                for mb in range(mbn):
                    pss = [
                        pp2.tile([P, N_HALF], f32, tag="ps2", name=f"ps2{suffix}_{mb}_{i}")
                        for i in range(NH)
                    ]
                    for ko in range(KO):
                        for nh in range(NH):
                            nc.tensor.matmul(
                                pss[nh][:],
                                lhsT=AT[:, ko, mb * P : (mb + 1) * P],
                                rhs=w_sb["wc"][:, ko, nh * N_HALF : (nh + 1) * N_HALF],
                                start=(ko == 0),
                                stop=(ko == KO - 1),
                            )
                    for nh in range(NH):
                        ot = smallp.tile([P, N_HALF], f32, tag="osb")
                        if with_bias:
                            nc.vector.tensor_tensor(
                                ot[:],
                                pss[nh][:],
                                b_sb["bc"][:, nh * N_HALF : (nh + 1) * N_HALF],
                                OP.add,
                            )
                        else:
                            nc.scalar.activation(ot[:], pss[nh][:], AF.Copy)
                        nc.sync.dma_start(
                            out_d[
                                t0 + (mb0 + mb) * P : t0 + (mb0 + mb + 1) * P,
                                nh * N_HALF : (nh + 1) * N_HALF,
                            ],
                            ot[:],
                        )

            def chunk_body(c, split_tail=False):
                t0 = c * T_CHUNK
                xq_t = xinp.tile([P, KO, T_CHUNK], bf16, tag="xq")
                nc.sync.dma_start(xq_t[:], xq_r[:, :, t0 : t0 + T_CHUNK])
                xk_t = xinp.tile([P, KO, T_CHUNK], bf16, tag="xk")
                nc.sync.dma_start(xk_t[:], xk_r[:, :, t0 : t0 + T_CHUNK])
                xv_t = xinp.tile([P, KO, T_CHUNK], bf16, tag="xv")
                nc.sync.dma_start(xv_t[:], xv_r[:, :, t0 : t0 + T_CHUNK])

                Q = workp.tile([P, MB, D_MODEL], f32, tag="Q")
                K = workp.tile([P, MB, D_MODEL], f32, tag="K")
                V = workp.tile([P, MB, D_MODEL], f32, tag="V")

                proj(xq_t, w_sb["wq"], True, Q, b_sb.get("bq"))
                proj(xk_t, w_sb["wk"], True, K, b_sb.get("bk"))
                proj(xv_t, w_sb["wv"], False, V, b_sb.get("bv"))

                if split_tail:
                    for mb in range(MB):
                        post_block(t0, Q, K, V, mb, 1, f"s{mb}")
                else:
                    post_block(t0, Q, K, V, 0, MB, "")

            repeat_ctx = (
                tc.For_i(0, repeat, 1) if repeat > 1 else contextlib.nullcontext()
            )
            with repeat_ctx:
                for c in range(N_CHUNKS):
                    chunk_body(c, split_tail=(c == N_CHUNKS - 1))

    nc.compile()
    return nc


def _get_module(with_bias: bool, repeat: int = 1):
    key = (bool(with_bias), repeat)
    if key not in _module_cache:
        _module_cache[key] = _build_module(*key)
    return _module_cache[key]


def _prepare_in_maps(v, k, q, wq_w, wq_b, wk_w, wk_b, wv_w, wv_b, wc_w, wc_b):
    with_bias = any(np.any(np.asarray(b)) for b in (wq_b, wk_b, wv_b, wc_b))

    q2 = np.asarray(q, dtype=np.float32).reshape(TOKENS, D_MODEL)
    k2 = np.asarray(k, dtype=np.float32).reshape(TOKENS, D_MODEL)
    v2 = np.asarray(v, dtype=np.float32).reshape(TOKENS, D_MODEL)

    w16 = {
        "wq": np.ascontiguousarray(np.asarray(wq_w, np.float32)).astype(_BF16),
        "wk": np.ascontiguousarray(np.asarray(wk_w, np.float32)).astype(_BF16),
        "wv": np.ascontiguousarray(np.asarray(wv_w, np.float32)).astype(_BF16),
        "wc": np.ascontiguousarray(np.asarray(wc_w, np.float32)).astype(_BF16),
    }

    bias_maps = {}
    if with_bias:
        bias_maps = {
            "bq_rep": np.ascontiguousarray(
                np.broadcast_to(np.asarray(wq_b, np.float32), (P, D_MODEL))
            ),
            "bk_rep": np.ascontiguousarray(
                np.broadcast_to(np.asarray(wk_b, np.float32), (P, D_MODEL))
            ),
            "bv_rep": np.ascontiguousarray(
                np.broadcast_to(np.asarray(wv_b, np.float32), (P, D_MODEL))
            ),
            "bc_rep": np.ascontiguousarray(
                np.broadcast_to(np.asarray(wc_b, np.float32), (P, D_MODEL))
            ),
        }

    in_maps = []
    for c in range(N_CORES):
        s = slice(c * TOK_PER_CORE, (c + 1) * TOK_PER_CORE)
        m = {
            "xq_t": np.ascontiguousarray(q2[s].T).astype(_BF16),
            "xk_t": np.ascontiguousarray(k2[s].T).astype(_BF16),
            "xv_t": np.ascontiguousarray(v2[s].T).astype(_BF16),
            **w16,
            **bias_maps,
        }
        in_maps.append(m)
    return in_maps, with_bias


def _assemble(results):
    out = np.empty((TOKENS, D_MODEL), np.float32)
    for c in range(N_CORES):
        out[c * TOK_PER_CORE : (c + 1) * TOK_PER_CORE] = results[c]["out_t"]
    return out.reshape(B, L, D_MODEL)


def run_kernel_raw(trace=False, **inputs):
    """Run on the 8 NeuronCores; returns (output, BassKernelResults)."""
    from concourse.bass_utils import run_bass_kernel_spmd

    in_maps, with_bias = _prepare_in_maps(**inputs)
    nc = _get_module(with_bias)
    res = run_bass_kernel_spmd(nc, in_maps, core_ids=list(range(N_CORES)), trace=trace)
    return _assemble(res.results), res


def kernel(**inputs):
    out, _ = run_kernel_raw(trace=False, **inputs)
    return out

